# revision 3
# baseline (speedup 1.0000x reference)
"""Trainium kernel for GraphVO: CNN backbone + 2 GraphConv layers + pose heads.

Strategy (per sharding hint):
  - Data-parallel CNN over nodes: 1024 nodes -> 128 per core on 8 NeuronCores.
  - Graph stage: partition edges by destination node (each core owns 128 dst
    nodes); source features obtained via on-device all_gather of the
    node-feature matrix. Host pre-builds a padded per-dst adjacency
    (src lists + mask) so segment softmax/sum become dense ops on device.
  - Heads computed on the dst-shard, outputs concatenated -> [1024, 7].

Everything runs SPMD on the 8 NeuronCores through a single jitted shard_map.
Host work is limited to weight folding (BN eval constants) and edge
bucketing/padding, i.e. sharding preparation.
"""

import numpy as np
import jax
import jax.numpy as jnp
from jax import lax
from jax.sharding import Mesh, PartitionSpec as P
from functools import partial

try:  # jax version compat
    from jax.experimental.shard_map import shard_map
except ImportError:
    from jax.shard_map import shard_map

EPS_BN = 1e-5
N_NODES = 1024
N_EDGES = 16384
N_CORES = 8
CNN_SPEC = [(7, 2, 3), (5, 2, 2), (5, 2, 2), (3, 1, 1), (3, 2, 1), (3, 1, 1), (3, 2, 1), (3, 1, 1)]
POOL_AFTER = {0, 1, 3, 5, 7}

_CACHE = {}
_LAST_ARGS = None


def _leaky(x, slope):
    return jnp.where(x >= 0, x, slope * x)


def _core_fn(xc, adjc, maskc, degc, *flat_w):
    """Runs on ONE NeuronCore (inside shard_map).

    xc:   [128, 1, 64, 64] local node images
    adjc: [128, DEG_PAD] int32 padded src ids for local dst nodes
    maskc:[128, DEG_PAD] f32 1/0 validity
    degc: [128, 1] f32 (unused placeholder, kept for clarity)
    flat_w: folded conv (w,b) x8, then graph weights.
    """
    it = iter(flat_w)
    conv_wb = [(next(it), next(it)) for _ in range(8)]
    (rel1_w, rel1_b, root1_w, rel2_w, rel2_b, root2_w,
     pos_w, pos_b, rot_w, rot_b) = [next(it) for _ in range(10)]

    # ---- CNN backbone (data parallel over the 128 local nodes) ----
    h = xc
    for i, (k, s, p) in enumerate(CNN_SPEC):
        w, b = conv_wb[i]
        h = lax.conv_general_dilated(
            h, w, (s, s), [(p, p), (p, p)],
            dimension_numbers=("NCHW", "OIHW", "NCHW"))
        h = h + b[None, :, None, None]
        h = _leaky(h, 0.1)
        if i in POOL_AFTER:
            h = lax.reduce_window(h, -jnp.inf, lax.max, (1, 1, 3, 3), (1, 1, 2, 2),
                                  [(0, 0), (0, 0), (1, 1), (1, 1)])
    feat = h.reshape(-1, 512)                         # [128, 512]

    # ---- gather features of ALL nodes (the graph is global) ----
    featf = lax.all_gather(feat, "c", axis=0, tiled=True)   # [1024, 512]
    cidx = lax.axis_index("c")
    node_idx = jnp.arange(N_NODES, dtype=featf.dtype)[:, None]
    x1 = jnp.concatenate([featf, node_idx], axis=1)   # [1024, 513]

    # ---- GraphConv 1: per-feature softmax aggregation over incoming edges ----
    m = x1[adjc]                                      # [128, DEG, 513]
    mk = maskc[:, :, None]
    neg = jnp.float32(-jnp.inf)
    mmax = jnp.max(jnp.where(mk > 0, m, neg), axis=1)             # [128, 513]
    mmax = jnp.where(jnp.isfinite(mmax), mmax, 0.0)
    e = jnp.exp(m - mmax[:, None, :]) * mk
    ssum = jnp.sum(e, axis=1)                                      # [128, 513]
    alpha = e / (ssum[:, None, :] + 1e-16)
    agg1 = jnp.sum(m * alpha, axis=1)                              # [128, 513]

    x1_loc = lax.dynamic_slice(x1, (cidx * 128, 0), (128, 513))
    h1 = _leaky(agg1 @ rel1_w + rel1_b + x1_loc @ root1_w, 0.01)   # [128, 256]

    # ---- GraphConv 2: sum aggregation ----
    h1f = lax.all_gather(h1, "c", axis=0, tiled=True)              # [1024, 256]
    x2 = jnp.concatenate([h1f, node_idx], axis=1)                  # [1024, 257]
    m2 = x2[adjc] * mk                                             # [128, DEG, 257]
    agg2 = jnp.sum(m2, axis=1)                                     # [128, 257]
    x2_loc = lax.dynamic_slice(x2, (cidx * 128, 0), (128, 257))
    h2 = _leaky(agg2 @ rel2_w + rel2_b + x2_loc @ root2_w, 0.01)   # [128, 64]

    # ---- heads ----
    ids_loc = lax.dynamic_slice(node_idx, (cidx * 128, 0), (128, 1))
    x3 = jnp.concatenate([h2, ids_loc.astype(h2.dtype)], axis=1)   # [128, 65]
    pos = x3 @ pos_w + pos_b
    rot = x3 @ rot_w + rot_b
    rot = rot / jnp.maximum(jnp.linalg.norm(rot, axis=1, keepdims=True), 1e-12)
    return jnp.concatenate([pos, rot], axis=1)                     # [128, 7]


def _build(deg_pad):
    devices = jax.devices()[:N_CORES]
    mesh = Mesh(np.asarray(devices), ("c",))
    n_w = 8 * 2 + 10
    fn = shard_map(
        _core_fn, mesh=mesh,
        in_specs=(P("c"), P("c"), P("c"), P("c")) + (P(),) * n_w,
        out_specs=P("c"), check_rep=False)
    return jax.jit(fn)


def kernel(x, edge_index, cnn_params, rel1_w, rel1_b, root1_w,
           rel2_w, rel2_b, root2_w, pos_w, pos_b, rot_w, rot_b):
    x = np.asarray(x, np.float32).reshape(N_NODES, 1, 64, 64)
    ei = np.asarray(edge_index)
    src = ei[0].astype(np.int64)
    dst = ei[1].astype(np.int64)

    # ---- host prep: fold eval-mode BN into conv weights ----
    folded = []
    for (w, b, g, be) in cnn_params:
        w = np.asarray(w, np.float32)
        b = np.asarray(b, np.float32)
        g = np.asarray(g, np.float32)
        be = np.asarray(be, np.float32)
        s = g / np.sqrt(1.0 + EPS_BN)
        folded.append((w * s[:, None, None, None], b * s + be))

    # ---- host prep: bucket edges by destination, pad to fixed degree ----
    deg = np.bincount(dst, minlength=N_NODES)
    deg_pad = int(((deg.max() + 7) // 8) * 8)
    order = np.argsort(dst, kind="stable")
    src_sorted = src[order]
    adj = np.zeros((N_NODES, deg_pad), np.int32)
    mask = np.zeros((N_NODES, deg_pad), np.float32)
    offs = np.zeros(N_NODES + 1, np.int64)
    np.cumsum(deg, out=offs[1:])
    cols = np.arange(len(src_sorted)) - offs[dst[order].astype(np.int64)]
    rows = dst[order].astype(np.int64)
    adj[rows, cols] = src_sorted.astype(np.int32)
    mask[rows, cols] = 1.0
    degf = deg.astype(np.float32)[:, None]

    key = deg_pad
    if key not in _CACHE:
        _CACHE[key] = _build(deg_pad)
    fn = _CACHE[key]

    flat_w = []
    for w, b in folded:
        flat_w += [w, b]
    flat_w += [np.asarray(a, np.float32) for a in
               (rel1_w, rel1_b, root1_w, rel2_w, rel2_b, root2_w,
                pos_w, pos_b, rot_w, rot_b)]

    args = (x, adj, mask, degf) + tuple(flat_w)
    out = fn(*args)
    out = np.asarray(jax.block_until_ready(out), np.float32)
    global _LAST_ARGS
    _LAST_ARGS = (fn, args)
    return out


# revision 5
# speedup vs baseline: 46.1684x; 46.1684x over previous
"""Trainium kernel for GraphVO: CNN backbone + 2 GraphConv layers + pose heads.

Strategy (per sharding hint):
  - Data-parallel CNN over nodes: 1024 nodes -> 128 per core on 8 NeuronCores.
  - Graph stage: partition edges by destination node (each core owns 128 dst
    nodes); source features obtained via on-device all_gather of the
    node-feature matrix. Host pre-builds a padded per-dst adjacency
    (src lists + mask) so segment softmax/sum become dense ops on device.
  - Heads computed on the dst-shard, outputs concatenated -> [1024, 7].

Everything runs SPMD on the 8 NeuronCores through a single jitted shard_map.
Host work is limited to weight folding (BN eval constants) and edge
bucketing/padding, i.e. sharding preparation.
"""

import numpy as np
import jax
import jax.numpy as jnp
from jax import lax
from jax.sharding import Mesh, PartitionSpec as P
from functools import partial

try:  # jax version compat
    from jax.experimental.shard_map import shard_map
except ImportError:
    from jax.shard_map import shard_map

EPS_BN = 1e-5
N_NODES = 1024
N_EDGES = 16384
N_CORES = 8
CNN_SPEC = [(7, 2, 3), (5, 2, 2), (5, 2, 2), (3, 1, 1), (3, 2, 1), (3, 1, 1), (3, 2, 1), (3, 1, 1)]
POOL_AFTER = {0, 1, 3, 5, 7}

_CACHE = {}
_LAST_ARGS = None


def _leaky(x, slope):
    return jnp.where(x >= 0, x, slope * x)


def _core_fn(xc, adjc, maskc, degc, *flat_w):
    """Runs on ONE NeuronCore (inside shard_map).

    xc:   [128, 1, 64, 64] local node images
    adjc: [128, DEG_PAD] int32 padded src ids for local dst nodes
    maskc:[128, DEG_PAD] f32 1/0 validity
    degc: [128, 1] f32 (unused placeholder, kept for clarity)
    flat_w: folded conv (w,b) x8, then graph weights.
    """
    it = iter(flat_w)
    conv_wb = [(next(it), next(it)) for _ in range(8)]
    (rel1_w, rel1_b, root1_w, rel2_w, rel2_b, root2_w,
     pos_w, pos_b, rot_w, rot_b) = [next(it) for _ in range(10)]

    # ---- CNN backbone (data parallel over the 128 local nodes) ----
    h = xc
    for i, (k, s, p) in enumerate(CNN_SPEC):
        w, b = conv_wb[i]
        h = lax.conv_general_dilated(
            h, w, (s, s), [(p, p), (p, p)],
            dimension_numbers=("NCHW", "OIHW", "NCHW"))
        h = h + b[None, :, None, None]
        h = _leaky(h, 0.1)
        if i in POOL_AFTER:
            h = lax.reduce_window(h, -jnp.inf, lax.max, (1, 1, 3, 3), (1, 1, 2, 2),
                                  [(0, 0), (0, 0), (1, 1), (1, 1)])
    feat = h.reshape(-1, 512)                         # [128, 512]

    # ---- gather features of ALL nodes (the graph is global) ----
    featf = lax.all_gather(feat, "c", axis=0, tiled=True)   # [1024, 512]
    cidx = lax.axis_index("c")
    node_idx = jnp.arange(N_NODES, dtype=featf.dtype)[:, None]
    x1 = jnp.concatenate([featf, node_idx], axis=1)   # [1024, 513]

    # ---- GraphConv 1: per-feature softmax aggregation over incoming edges ----
    m = x1[adjc]                                      # [128, DEG, 513]
    mk = maskc[:, :, None]
    neg = jnp.float32(-jnp.inf)
    mmax = jnp.max(jnp.where(mk > 0, m, neg), axis=1)             # [128, 513]
    mmax = jnp.where(jnp.isfinite(mmax), mmax, 0.0)
    e = jnp.exp(m - mmax[:, None, :]) * mk
    ssum = jnp.sum(e, axis=1)                                      # [128, 513]
    alpha = e / (ssum[:, None, :] + 1e-16)
    agg1 = jnp.sum(m * alpha, axis=1)                              # [128, 513]

    x1_loc = lax.dynamic_slice(x1, (cidx * 128, 0), (128, 513))
    h1 = _leaky(agg1 @ rel1_w + rel1_b + x1_loc @ root1_w, 0.01)   # [128, 256]

    # ---- GraphConv 2: sum aggregation ----
    h1f = lax.all_gather(h1, "c", axis=0, tiled=True)              # [1024, 256]
    x2 = jnp.concatenate([h1f, node_idx], axis=1)                  # [1024, 257]
    m2 = x2[adjc] * mk                                             # [128, DEG, 257]
    agg2 = jnp.sum(m2, axis=1)                                     # [128, 257]
    x2_loc = lax.dynamic_slice(x2, (cidx * 128, 0), (128, 257))
    h2 = _leaky(agg2 @ rel2_w + rel2_b + x2_loc @ root2_w, 0.01)   # [128, 64]

    # ---- heads ----
    ids_loc = lax.dynamic_slice(node_idx, (cidx * 128, 0), (128, 1))
    x3 = jnp.concatenate([h2, ids_loc.astype(h2.dtype)], axis=1)   # [128, 65]
    pos = x3 @ pos_w + pos_b
    rot = x3 @ rot_w + rot_b
    rot = rot / jnp.maximum(jnp.linalg.norm(rot, axis=1, keepdims=True), 1e-12)
    return jnp.concatenate([pos, rot], axis=1)                     # [128, 7]


def _build(deg_pad):
    devices = jax.devices()[:N_CORES]
    mesh = Mesh(np.asarray(devices), ("c",))
    n_w = 8 * 2 + 10
    fn = shard_map(
        _core_fn, mesh=mesh,
        in_specs=(P("c"), P("c"), P("c"), P("c")) + (P(),) * n_w,
        out_specs=P("c"), check_rep=False)
    from jax.sharding import NamedSharding
    shardings = (NamedSharding(mesh, P("c")),) * 4 + (NamedSharding(mesh, P()),) * n_w
    return jax.jit(fn), shardings


def kernel(x, edge_index, cnn_params, rel1_w, rel1_b, root1_w,
           rel2_w, rel2_b, root2_w, pos_w, pos_b, rot_w, rot_b):
    x = np.asarray(x, np.float32).reshape(N_NODES, 1, 64, 64)
    ei = np.asarray(edge_index)
    src = ei[0].astype(np.int64)
    dst = ei[1].astype(np.int64)

    # ---- host prep: fold eval-mode BN into conv weights ----
    folded = []
    for (w, b, g, be) in cnn_params:
        w = np.asarray(w, np.float32)
        b = np.asarray(b, np.float32)
        g = np.asarray(g, np.float32)
        be = np.asarray(be, np.float32)
        s = g / np.sqrt(1.0 + EPS_BN)
        folded.append((w * s[:, None, None, None], b * s + be))

    # ---- host prep: bucket edges by destination, pad to fixed degree ----
    deg = np.bincount(dst, minlength=N_NODES)
    deg_pad = int(((deg.max() + 7) // 8) * 8)
    order = np.argsort(dst, kind="stable")
    src_sorted = src[order]
    adj = np.zeros((N_NODES, deg_pad), np.int32)
    mask = np.zeros((N_NODES, deg_pad), np.float32)
    offs = np.zeros(N_NODES + 1, np.int64)
    np.cumsum(deg, out=offs[1:])
    cols = np.arange(len(src_sorted)) - offs[dst[order].astype(np.int64)]
    rows = dst[order].astype(np.int64)
    adj[rows, cols] = src_sorted.astype(np.int32)
    mask[rows, cols] = 1.0
    degf = deg.astype(np.float32)[:, None]

    key = deg_pad
    if key not in _CACHE:
        _CACHE[key] = _build(deg_pad)
    fn, shardings = _CACHE[key]

    flat_w = []
    for w, b in folded:
        flat_w += [w, b]
    flat_w += [np.asarray(a, np.float32) for a in
               (rel1_w, rel1_b, root1_w, rel2_w, rel2_b, root2_w,
                pos_w, pos_b, rot_w, rot_b)]

    args = (x, adj, mask, degf) + tuple(flat_w)
    dargs = jax.device_put(args, shardings)
    out = fn(*dargs)
    out = np.asarray(jax.block_until_ready(out), np.float32)
    global _LAST_ARGS
    _LAST_ARGS = (fn, dargs)
    return out
